# revision 37
# baseline (speedup 1.0000x reference)
"""CSI decoder kernel: LayerNorm(d) -> Linear(512->2) -> per-subcarrier scale -> complex.

Distribution: pure data parallel over 8 NeuronCores. The flattened token axis
(B*A_bs*A_ue*S = 262144 tokens) is split contiguously into 8 shards of 32768
tokens; each core reads its 64 MiB x-shard and produces [32768, 2] f32
(real, imag interleaved), gathered host-side into the complex64 output.

Math: the normalized tensor is never materialized. With
  Wg[o] = W[o] * gamma,  sW[o] = sum(Wg[o]),  c[o] = W[o] @ beta + b[o]
each token needs only 4 reductions over d:
  s1 = sum(x), s2 = sum(x^2), p_o = x . Wg[o]
  mu = s1/512, var = s2/512 - mu^2, rstd = 1/sqrt(var+eps)
  cf_o = rstd*(p_o - mu*sW[o]) + c[o];  out = cf * (|scalers_s| + 0.1)

Design (measured-cost driven, v8):
  DMA       SWDGE (gpsimd) input DMAs cast f32 -> bf16 in flight: the 64 MiB
            HBM read is unchanged (that is the roofline) but SBUF traffic,
            footprint and every on-chip stage drop a perf tier.
  TensorE   bf16 transposes to bf16 PSUM (1 bank / 4-tile quad), then
            [d x tok]^T @ [d x 3] bf16 matmuls accumulate p0, p1, s1 into
            f32 PSUM stats slots with tokens on partitions.          ~124us
  VectorE   drains transposed bf16 PSUM quads -> SBUF (tensor_copy in the
            2x_1P packed mode), 6/16 of sum(x^2) via stt-accumulate, and
            the batched epilogue.                                    ~155us
  ScalarE   10/16 of sum(x^2) via Square activation w/ accumulator.  ~160us
Scheduling: each quad's matmuls are emitted AFTER the next quad's transposes
(software pipelining) so TensorE's in-order queue never waits on the DVE
drain.  The epilogue is ALL-DVE: rstd comes from a Quake-style bit-trick
seed + two Newton steps instead of ScalarE Sqrt + iterative reciprocal, so
no cross-engine wait can be hoisted in front of ScalarE's square stream by
the tile scheduler (which moves ops ~2 chunks earlier than emission and
previously cost 10-25us per epilogue).  Epilogue 0 is emitted at chunk 12,
epilogue 1 split by columns (96 at chunk 14, the last 32 after the final
flush); the last two chunks shift s2 toward ScalarE since DVE owns the tail.
HBM is shared per NC pair (~716 GB/s/stack), so the pair's 128 MiB read
bounds the kernel at ~188us + ramp + tail regardless of per-core burst
rates; measured exec varies 210-230us with the cores' stagger draw.

On-chip layout: within a core's 32768-token shard, token t maps to
(partition p, column j) = (t // 256, t % 256), so the input and output DMAs
see large contiguous per-partition runs.
"""

from contextlib import ExitStack

import ml_dtypes
import numpy as np

import concourse.bass as bass
import concourse.tile as tile
from concourse import mybir
from concourse.bass_utils import run_bass_kernel_spmd

N_CORES = 8
B, A_BS, A_UE, S, D = 16, 64, 4, 64, 512
TOKENS = B * A_BS * A_UE * S            # 262144
TOK_PER_CORE = TOKENS // N_CORES        # 32768
NTILES = TOK_PER_CORE // 128            # 256 token-tiles of [128, 512]
CHUNK = 16                              # token-tiles per chunk
NCHUNKS = NTILES // CHUNK               # 16
EPS = 1e-5
NDCH = D // 128                         # 4 d-chunks per token-tile
DVE_S2_TILES = 6                        # tiles 0..5 of each chunk: s2 on DVE
QUAD = 4                                # token-tiles per PSUM drain batch

F32 = mybir.dt.float32
BF16 = mybir.dt.bfloat16
ALU = mybir.AluOpType
AF = mybir.ActivationFunctionType


def _split_multi_waits(nc):
    """Workaround for this walrus build: an instruction may carry at most one
    embedded sync wait; hoist extras into standalone no-ops placed before it."""
    for bbobj in nc.bb_map.values():
        insts = list(bbobj.bb.instructions)
        changed = False
        new_list = []
        for inst in insts:
            si = inst.sync_info
            if si is not None and si.on_wait is not None and len(si.on_wait) > 1:
                waits = list(si.on_wait)
                si.on_wait = waits[:1]
                for i, w in enumerate(waits[1:]):
                    nop = mybir.InstNoOp(name=f"wsplit_{inst.name}_{i}")
                    nop.engine = inst.engine
                    nop.sync_info = mybir.SyncInfo(on_wait=[w], on_update=[])
                    try:
                        nc.register_instruction(nop, overwrite=True)
                    except Exception:
                        pass
                    new_list.append(nop)
                changed = True
            new_list.append(inst)
        if changed:
            bbobj.bb.instructions = new_list


def _build(sw0: float, sw1: float, c0: float, c1: float):
    nc = bass.Bass(
        "TRN2", target_bir_lowering=False, debug=False, num_devices=N_CORES
    )
    x_in = nc.dram_tensor("x", [TOK_PER_CORE, D], F32, kind="ExternalInput")
    ident_in = nc.dram_tensor("ident", [128, 128], BF16, kind="ExternalInput")
    wst_in = nc.dram_tensor("wst", [128, NDCH, 4], BF16, kind="ExternalInput")
    sc_in = nc.dram_tensor("sc", [128, NTILES], F32, kind="ExternalInput")
    out_t = nc.dram_tensor("out", [TOK_PER_CORE, 2], F32, kind="ExternalOutput")

    # token t = p*NTILES + j  ->  [partition, tile-column, d]
    x_v = x_in.ap().rearrange("(p j) d -> p j d", p=128)
    out_v = out_t.ap().rearrange("(p j) two -> p j two", p=128)

    with tile.TileContext(nc) as tc, ExitStack() as ctx:
        const_pool = ctx.enter_context(tc.tile_pool(name="const", bufs=1))
        xf_pool = ctx.enter_context(tc.tile_pool(name="xf", bufs=9))
        xtb_pool = ctx.enter_context(tc.tile_pool(name="xtb", bufs=6))
        ps_pool = ctx.enter_context(tc.tile_pool(name="ps", bufs=3, space="PSUM"))
        st_pool = ctx.enter_context(tc.tile_pool(name="st", bufs=1, space="PSUM"))
        stat_pool = ctx.enter_context(tc.tile_pool(name="stat", bufs=1))
        ep_pool = ctx.enter_context(tc.tile_pool(name="ep", bufs=1))

        ident = const_pool.tile([128, 128], BF16)
        nc.sync.dma_start(out=ident[:], in_=ident_in.ap())
        wst = const_pool.tile([128, NDCH, 4], BF16)
        nc.scalar.dma_start(out=wst[:], in_=wst_in.ap())
        sc = const_pool.tile([128, NTILES], F32)
        nc.scalar.dma_start(out=sc[:], in_=sc_in.ap())

        # stats PSUM: one bank per 128 token-tiles; slot j%128 is 4 wide
        stats_ps = [
            st_pool.tile([128, 512], F32, name=f"stats_ps{h}", tag=f"stats_ps{h}")
            for h in range(NTILES // 128)
        ]

        s2B_dve = [
            stat_pool.tile([128, 128], F32, name=f"s2d{h}", tag=f"s2d{h}")
            for h in range(NTILES // 128)
        ]
        s2B_act = [
            stat_pool.tile([128, 128], F32, name=f"s2a{h}", tag=f"s2a{h}")
            for h in range(NTILES // 128)
        ]
        dve_junk = stat_pool.tile([128, D], BF16)
        act_junk = stat_pool.tile([128, D], BF16)

        for h in range(NTILES // 128):
            nc.vector.memset(s2B_dve[h][:], 0.0)
            nc.scalar.memzero(s2B_act[h][:])
        # pre-load the Sqrt activation table so the one-time ~2.7us table
        # fetch overlaps the first input DMA instead of stalling the epilogue
        sqrt_warm = stat_pool.tile([128, 1], F32)
        nc.scalar.memzero(sqrt_warm[:])
        nc.scalar.activation(out=sqrt_warm[:], in_=sqrt_warm[:], func=AF.Sqrt)

        # The epilogue for a column range [c0, c1) of stats group h is emitted
        # in three phases at well-separated points of the chunk loop so that
        # by the time each engine's in-order queue reaches a phase, its inputs
        # have long been produced by the other engines — no queue stalls.
        ep_state = {}

        def ep_dve1(h, a0, a1, tg):
            w = a1 - a0
            stats_sb = ep_pool.tile([128, w * 4], F32, tag=f"stats_sb{tg}")
            nc.vector.tensor_copy(stats_sb[:], stats_ps[h][:, a0 * 4 : a1 * 4])
            quad = stats_sb[:].rearrange("p (j four) -> p j four", four=4)
            mu = ep_pool.tile([128, w], F32, tag=f"mu{tg}")
            nc.vector.tensor_scalar_mul(mu[:], quad[:, :, 2], 1.0 / D)
            # s2 = s2_dve + s2_act (disjoint columns, zero elsewhere)
            s2h = ep_pool.tile([128, w], F32, tag=f"s2h{tg}")
            nc.vector.tensor_tensor(
                out=s2h[:], in0=s2B_dve[h][:, a0:a1], in1=s2B_act[h][:, a0:a1],
                op=ALU.add,
            )
            # hvar = (s2/512 + eps - mu^2) / 2  -- the 1/2 folded in for the
            # Newton rsqrt iteration below
            ex2h = ep_pool.tile([128, w], F32, tag=f"ex2{tg}")
            nc.vector.tensor_scalar(
                out=ex2h[:], in0=s2h[:], scalar1=0.5 / D, scalar2=0.5 * EPS,
                op0=ALU.mult, op1=ALU.add,
            )
            nmusqh = ep_pool.tile([128, w], F32, tag=f"nmusq{tg}")
            nc.vector.scalar_tensor_tensor(
                out=nmusqh[:], in0=mu[:], scalar=-0.5, in1=mu[:],
                op0=ALU.mult, op1=ALU.mult,
            )
            hvar = ep_pool.tile([128, w], F32, tag=f"var{tg}")
            nc.vector.tensor_tensor(out=hvar[:], in0=nmusqh[:], in1=ex2h[:], op=ALU.add)
            ep_state[tg] = (quad, mu, hvar)

        # rstd = 1/sqrt(2*hvar) entirely on DVE: Quake-style bit seed (magic
        # shifted by 2^22 because we seed from var/2 bits) + two Newton steps
        # y <- y*(1.5 - hvar*y^2); rel err ~2e-5.  No ScalarE involvement, so
        # no cross-engine wait can ever land in front of later squares.
        RSQRT_MAGIC = 0x5F3759DF - 0x00400000
        I32 = mybir.dt.int32

        def ep_rsqrt(tg):
            quad, mu, hvar = ep_state[tg]
            w = hvar.shape[1]
            t1 = ep_pool.tile([128, w], I32, tag=f"rsq_t1{tg}")
            nc.vector.tensor_scalar(
                out=t1[:], in0=hvar[:].bitcast(I32), scalar1=1, scalar2=None,
                op0=ALU.arith_shift_right,
            )
            y = ep_pool.tile([128, w], F32, tag=f"rsq_y{tg}")
            nc.vector.tensor_scalar(
                out=y[:].bitcast(I32), in0=t1[:], scalar1=-1,
                scalar2=RSQRT_MAGIC, op0=ALU.mult, op1=ALU.add,
            )
            for it in range(2):
                y2 = ep_pool.tile([128, w], F32, tag=f"rsq_y2{tg}")
                nc.vector.tensor_tensor(out=y2[:], in0=y[:], in1=y[:], op=ALU.mult)
                nc.vector.tensor_tensor(out=y2[:], in0=y2[:], in1=hvar[:], op=ALU.mult)
                nc.vector.tensor_scalar(
                    out=y2[:], in0=y2[:], scalar1=-1.0, scalar2=1.5,
                    op0=ALU.mult, op1=ALU.add,
                )
                yn = ep_pool.tile([128, w], F32, tag=f"rsq_y{it}{tg}")
                nc.vector.tensor_tensor(out=yn[:], in0=y[:], in1=y2[:], op=ALU.mult)
                y = yn
            ep_state[tg] = (quad, mu, y)

        def ep_dve2(h, a0, a1, tg):
            quad, mu, rstd = ep_state.pop(tg)
            w = a1 - a0
            hs = slice(h * 128 + a0, h * 128 + a1)
            outB = ep_pool.tile([128, w, 2], F32, tag=f"outB{tg}")
            for o, (sw, c) in enumerate(((sw0, c0), (sw1, c1))):
                a = ep_pool.tile([128, w], F32, tag=f"ep_a{tg}")
                # a = p - mu*sW
                nc.vector.scalar_tensor_tensor(
                    out=a[:], in0=mu[:], scalar=-sw, in1=quad[:, :, o],
                    op0=ALU.mult, op1=ALU.add,
                )
                cf = ep_pool.tile([128, w], F32, tag=f"ep_cf{tg}")
                nc.vector.tensor_tensor(
                    out=cf[:], in0=a[:], in1=rstd[:], op=ALU.mult
                )
                # out = (cf + c) * scale   (scale varies along the free axis)
                nc.vector.scalar_tensor_tensor(
                    out=outB[:, :, o], in0=cf[:], scalar=c, in1=sc[:, hs],
                    op0=ALU.add, op1=ALU.mult,
                )
            nc.sync.dma_start(out=out_v[:, hs, :], in_=outB[:])

        def emit_matmuls(j0, xtb):
            """stats matmuls for the quad whose first tile-index is j0."""
            for k in range(QUAD):
                j = j0 + k
                slot = stats_ps[j // 128][:, (j % 128) * 4 : (j % 128) * 4 + 3]
                for c in range(NDCH):
                    nc.tensor.matmul(
                        out=slot,
                        lhsT=xtb[:, k, c, :],
                        rhs=wst[:, c, 0:3],
                        start=(c == 0),
                        stop=(c == NDCH - 1),
                    )

        # Software-pipelined emission: each quad's matmuls are deferred until
        # after the NEXT quad's transposes, so TensorE's in-order queue never
        # sits in a transposes -> (wait for DVE drain) -> matmuls chain; the
        # drain of quad q overlaps the transposes of quad q+1.
        pending = None  # (j0, xtb) of the quad whose matmuls are not yet emitted
        for ci in range(NCHUNKS):
            xf = xf_pool.tile([128, CHUNK, D], BF16)
            if ci == 0:
                qsplit = (2, 4, 4, 6)
            elif ci <= 2 or ci == NCHUNKS - 1:
                # finer DMAs while the HBM pair-contention phase is slow
                # (ramp) and for the last chunk (tail): compute on each
                # quarter starts as soon as that quarter lands
                qsplit = (4, 4, 4, 4)
            else:
                qsplit = (8, 8)
            lo = 0
            for qs in qsplit:
                nc.gpsimd.dma_start(
                    out=xf[:, lo : lo + qs, :],
                    in_=x_v[:, ci * CHUNK + lo : ci * CHUNK + lo + qs, :],
                )
                lo += qs

            for tp in range(CHUNK // QUAD):
                # four token-tiles share one 2-bank bf16 PSUM tile; DVE drains
                # them with a single [128, 2048] 2x-mode tensor_copy
                xtp = ps_pool.tile([128, QUAD, NDCH, 128], BF16)
                xtb = xtb_pool.tile([128, QUAD, NDCH, 128], BF16)
                for k in range(QUAD):
                    t = tp * QUAD + k
                    j = ci * CHUNK + t
                    xt = xf[:, t, :]
                    for c in range(NDCH):
                        nc.tensor.transpose(
                            out=xtp[:, k, c, :],
                            in_=xt[:, c * 128 : (c + 1) * 128],
                            identity=ident[:],
                        )
                    # sum(x^2) needs only xf: emit alongside the transposes.
                    # Chunk 0 gives ScalarE the EARLY tiles so its stream
                    # starts with the first sub-DMA (~4us ramp saving); the
                    # last two chunks shift s2 toward ScalarE since DVE owns
                    # the epilogue-1 tail while ScalarE finishes ~20us early.
                    if ci == 0:
                        on_scalar = t < 10
                    elif ci >= NCHUNKS - 2:
                        on_scalar = t >= 3
                    else:
                        on_scalar = t >= DVE_S2_TILES
                    if on_scalar:
                        nc.scalar.activation(
                            out=act_junk[:], in_=xt, func=AF.Square,
                            accum_out=s2B_act[j // 128][:, j % 128 : j % 128 + 1],
                        )
                    else:
                        nc.vector.scalar_tensor_tensor(
                            out=dve_junk[:], in0=xt,
                            scalar=1.0, in1=xt, op0=ALU.mult, op1=ALU.mult,
                            accum_out=s2B_dve[j // 128][:, j % 128 : j % 128 + 1],
                        )
                nc.vector.tensor_copy(xtb[:], xtp[:])
                if pending is not None:
                    emit_matmuls(*pending)
                pending = (ci * CHUNK + tp * QUAD, xtb)
            # Epilogue 0 (chunks 0-7) is emitted after chunk 12, well into the
            # post-contention phase where DVE runs ahead of ScalarE and all of
            # its inputs are 4+ chunks stale — the var-dependent Sqrt in
            # ScalarE's in-order queue then waits ~1us instead of ~20us (which
            # at chunk 8, during the HBM pair-contention phase, only got
            # absorbed on lucky runs).  Epilogue 1 runs entirely at the end;
            # any earlier placement puts its Sqrt in front of the last chunks'
            # squares with fresh inputs and delays the tail (measured +19us).
            if ci == 12:
                ep_dve1(0, 0, 128, "A")
                ep_rsqrt("A")
                ep_dve2(0, 0, 128, "A")
            elif ci == 14:
                # epilogue 1a (cols 0-95 <- chunks 8-13, matmuls flushed at
                # chunk 14's first quad): all-DVE, so even scheduler hoisting
                # can only cost a small same-engine bubble, not a stall in
                # front of ScalarE's remaining squares
                ep_dve1(1, 0, 96, "B")
                ep_rsqrt("B")
                ep_dve2(1, 0, 96, "B")
            elif ci == NCHUNKS - 1:
                emit_matmuls(*pending)
                pending = None
                # short 32-column tail (chunks 14-15)
                ep_dve1(1, 96, 128, "C")
                ep_rsqrt("C")
                ep_dve2(1, 96, 128, "C")

    _split_multi_waits(nc)
    return nc


def _prepare(x, ln_gamma, ln_beta, W, b, scalers):
    x = np.asarray(x, dtype=np.float32)
    ln_gamma = np.asarray(ln_gamma, dtype=np.float32)
    ln_beta = np.asarray(ln_beta, dtype=np.float32)
    W = np.asarray(W, dtype=np.float32)
    b = np.asarray(b, dtype=np.float32)
    scalers = np.asarray(scalers, dtype=np.float32)

    wg = W * ln_gamma[None, :]                      # [2, 512]
    sw = wg.sum(axis=1)                             # [2]
    c = W @ ln_beta + b                             # [2]
    # wst[k, c, :] = (Wg0[c*128+k], Wg1[c*128+k], 1, 0)
    wst = np.zeros((128, NDCH, 4), dtype=np.float32)
    wst[:, :, 0] = wg[0].reshape(NDCH, 128).T
    wst[:, :, 1] = wg[1].reshape(NDCH, 128).T
    wst[:, :, 2] = 1.0
    wst = np.ascontiguousarray(wst.astype(ml_dtypes.bfloat16))
    ident = np.ascontiguousarray(np.eye(128, dtype=np.float32).astype(ml_dtypes.bfloat16))
    # token t = p*NTILES + j ; subcarrier s = t % 64 = j % 64 (NTILES % 64 == 0)
    scale = np.abs(scalers) + 0.1                   # [64]
    sc_row = scale[(np.arange(NTILES) % S)].astype(np.float32)
    sc_rep = np.ascontiguousarray(
        np.broadcast_to(sc_row[None, :], (128, NTILES))
    )

    nc = _build(float(sw[0]), float(sw[1]), float(c[0]), float(c[1]))

    x_flat = np.ascontiguousarray(x.reshape(TOKENS, D))
    in_maps = [
        {
            "x": x_flat[i * TOK_PER_CORE : (i + 1) * TOK_PER_CORE],
            "ident": ident,
            "wst": wst,
            "sc": sc_rep,
        }
        for i in range(N_CORES)
    ]
    return nc, in_maps


def kernel(x, ln_gamma, ln_beta, W, b, scalers):
    nc, in_maps = _prepare(x, ln_gamma, ln_beta, W, b, scalers)
    res = run_bass_kernel_spmd(nc, in_maps, core_ids=list(range(N_CORES)))
    out = np.concatenate([res.results[i]["out"] for i in range(N_CORES)], axis=0)
    out = np.ascontiguousarray(out.astype(np.float32))
    return out.view(np.complex64).reshape(B, A_BS, A_UE, S)


# revision 38
# speedup vs baseline: 1.1021x; 1.1021x over previous
"""CSI decoder kernel: LayerNorm(d) -> Linear(512->2) -> per-subcarrier scale -> complex.

Distribution: pure data parallel over 8 NeuronCores. The flattened token axis
(B*A_bs*A_ue*S = 262144 tokens) is split contiguously into 8 shards of 32768
tokens; each core reads its 64 MiB x-shard and produces [32768, 2] f32
(real, imag interleaved), gathered host-side into the complex64 output.

Math: the normalized tensor is never materialized. With
  Wg[o] = W[o] * gamma,  sW[o] = sum(Wg[o]),  c[o] = W[o] @ beta + b[o]
each token needs only 4 reductions over d:
  s1 = sum(x), s2 = sum(x^2), p_o = x . Wg[o]
  mu = s1/512, var = s2/512 - mu^2, rstd = 1/sqrt(var+eps)
  cf_o = rstd*(p_o - mu*sW[o]) + c[o];  out = cf * (|scalers_s| + 0.1)

Design (measured-cost driven, v8):
  DMA       SWDGE (gpsimd) input DMAs cast f32 -> bf16 in flight: the 64 MiB
            HBM read is unchanged (that is the roofline) but SBUF traffic,
            footprint and every on-chip stage drop a perf tier.
  TensorE   bf16 transposes to bf16 PSUM (1 bank / 4-tile quad), then
            [d x tok]^T @ [d x 3] bf16 matmuls accumulate p0, p1, s1 into
            f32 PSUM stats slots with tokens on partitions.          ~124us
  VectorE   drains transposed bf16 PSUM quads -> SBUF (tensor_copy in the
            2x_1P packed mode), 6/16 of sum(x^2) via stt-accumulate, and
            the batched epilogue.                                    ~155us
  ScalarE   10/16 of sum(x^2) via Square activation w/ accumulator.  ~160us
Scheduling: each quad's matmuls are emitted AFTER the next quad's transposes
(software pipelining) so TensorE's in-order queue never waits on the DVE
drain.  The epilogue is ALL-DVE: rstd comes from a Quake-style bit-trick
seed + two Newton steps instead of ScalarE Sqrt + iterative reciprocal, so
no cross-engine wait can be hoisted in front of ScalarE's square stream by
the tile scheduler (which moves ops ~2 chunks earlier than emission and
previously cost 10-25us per epilogue).  Epilogue 0 is emitted at chunk 12,
epilogue 1 split by columns (96 at chunk 14, the last 32 after the final
flush); the last two chunks shift s2 toward ScalarE since DVE owns the tail.
HBM is shared per NC pair (~716 GB/s/stack), so the pair's 128 MiB read
bounds the kernel at ~188us + ramp + tail regardless of per-core burst
rates; measured exec varies 210-230us with the cores' stagger draw.

On-chip layout: within a core's 32768-token shard, token t maps to
(partition p, column j) = (t // 256, t % 256), so the input and output DMAs
see large contiguous per-partition runs.
"""

from contextlib import ExitStack

import ml_dtypes
import numpy as np

import concourse.bass as bass
import concourse.tile as tile
from concourse import mybir
from concourse.bass_utils import run_bass_kernel_spmd

N_CORES = 8
B, A_BS, A_UE, S, D = 16, 64, 4, 64, 512
TOKENS = B * A_BS * A_UE * S            # 262144
TOK_PER_CORE = TOKENS // N_CORES        # 32768
NTILES = TOK_PER_CORE // 128            # 256 token-tiles of [128, 512]
CHUNK = 16                              # token-tiles per chunk
NCHUNKS = NTILES // CHUNK               # 16
EPS = 1e-5
NDCH = D // 128                         # 4 d-chunks per token-tile
DVE_S2_TILES = 6                        # tiles 0..5 of each chunk: s2 on DVE
QUAD = 4                                # token-tiles per PSUM drain batch

F32 = mybir.dt.float32
BF16 = mybir.dt.bfloat16
ALU = mybir.AluOpType
AF = mybir.ActivationFunctionType


def _split_multi_waits(nc):
    """Workaround for this walrus build: an instruction may carry at most one
    embedded sync wait; hoist extras into standalone no-ops placed before it."""
    for bbobj in nc.bb_map.values():
        insts = list(bbobj.bb.instructions)
        changed = False
        new_list = []
        for inst in insts:
            si = inst.sync_info
            if si is not None and si.on_wait is not None and len(si.on_wait) > 1:
                waits = list(si.on_wait)
                si.on_wait = waits[:1]
                for i, w in enumerate(waits[1:]):
                    nop = mybir.InstNoOp(name=f"wsplit_{inst.name}_{i}")
                    nop.engine = inst.engine
                    nop.sync_info = mybir.SyncInfo(on_wait=[w], on_update=[])
                    try:
                        nc.register_instruction(nop, overwrite=True)
                    except Exception:
                        pass
                    new_list.append(nop)
                changed = True
            new_list.append(inst)
        if changed:
            bbobj.bb.instructions = new_list


def _build(sw0: float, sw1: float, c0: float, c1: float):
    nc = bass.Bass(
        "TRN2", target_bir_lowering=False, debug=False, num_devices=N_CORES
    )
    x_in = nc.dram_tensor("x", [TOK_PER_CORE, D], F32, kind="ExternalInput")
    ident_in = nc.dram_tensor("ident", [128, 128], BF16, kind="ExternalInput")
    wst_in = nc.dram_tensor("wst", [128, NDCH, 4], BF16, kind="ExternalInput")
    sc_in = nc.dram_tensor("sc", [128, NTILES], F32, kind="ExternalInput")
    out_t = nc.dram_tensor("out", [TOK_PER_CORE, 2], F32, kind="ExternalOutput")

    # token t = p*NTILES + j  ->  [partition, tile-column, d]
    x_v = x_in.ap().rearrange("(p j) d -> p j d", p=128)
    out_v = out_t.ap().rearrange("(p j) two -> p j two", p=128)

    with tile.TileContext(nc) as tc, ExitStack() as ctx:
        const_pool = ctx.enter_context(tc.tile_pool(name="const", bufs=1))
        xf_pool = ctx.enter_context(tc.tile_pool(name="xf", bufs=9))
        xtb_pool = ctx.enter_context(tc.tile_pool(name="xtb", bufs=6))
        ps_pool = ctx.enter_context(tc.tile_pool(name="ps", bufs=3, space="PSUM"))
        st_pool = ctx.enter_context(tc.tile_pool(name="st", bufs=1, space="PSUM"))
        stat_pool = ctx.enter_context(tc.tile_pool(name="stat", bufs=1))
        ep_pool = ctx.enter_context(tc.tile_pool(name="ep", bufs=1))

        ident = const_pool.tile([128, 128], BF16)
        nc.sync.dma_start(out=ident[:], in_=ident_in.ap())
        wst = const_pool.tile([128, NDCH, 4], BF16)
        nc.scalar.dma_start(out=wst[:], in_=wst_in.ap())
        sc = const_pool.tile([128, NTILES], F32)
        nc.scalar.dma_start(out=sc[:], in_=sc_in.ap())

        # stats PSUM: one bank per 128 token-tiles; slot j%128 is 4 wide
        stats_ps = [
            st_pool.tile([128, 512], F32, name=f"stats_ps{h}", tag=f"stats_ps{h}")
            for h in range(NTILES // 128)
        ]

        s2B_dve = [
            stat_pool.tile([128, 128], F32, name=f"s2d{h}", tag=f"s2d{h}")
            for h in range(NTILES // 128)
        ]
        s2B_act = [
            stat_pool.tile([128, 128], F32, name=f"s2a{h}", tag=f"s2a{h}")
            for h in range(NTILES // 128)
        ]
        dve_junk = stat_pool.tile([128, D], BF16)
        act_junk = stat_pool.tile([128, D], BF16)

        for h in range(NTILES // 128):
            nc.vector.memset(s2B_dve[h][:], 0.0)
            nc.scalar.memzero(s2B_act[h][:])
        # pre-load the Sqrt activation table so the one-time ~2.7us table
        # fetch overlaps the first input DMA instead of stalling the epilogue
        sqrt_warm = stat_pool.tile([128, 1], F32)
        nc.scalar.memzero(sqrt_warm[:])
        nc.scalar.activation(out=sqrt_warm[:], in_=sqrt_warm[:], func=AF.Sqrt)

        # The epilogue for a column range [c0, c1) of stats group h is emitted
        # in three phases at well-separated points of the chunk loop so that
        # by the time each engine's in-order queue reaches a phase, its inputs
        # have long been produced by the other engines — no queue stalls.
        ep_state = {}

        def ep_dve1(h, a0, a1, tg):
            w = a1 - a0
            stats_sb = ep_pool.tile([128, w * 4], F32, tag=f"stats_sb{tg}")
            nc.vector.tensor_copy(stats_sb[:], stats_ps[h][:, a0 * 4 : a1 * 4])
            quad = stats_sb[:].rearrange("p (j four) -> p j four", four=4)
            mu = ep_pool.tile([128, w], F32, tag=f"mu{tg}")
            nc.vector.tensor_scalar_mul(mu[:], quad[:, :, 2], 1.0 / D)
            # s2 = s2_dve + s2_act (disjoint columns, zero elsewhere)
            s2h = ep_pool.tile([128, w], F32, tag=f"s2h{tg}")
            nc.vector.tensor_tensor(
                out=s2h[:], in0=s2B_dve[h][:, a0:a1], in1=s2B_act[h][:, a0:a1],
                op=ALU.add,
            )
            # hvar = (s2/512 + eps - mu^2) / 2  -- the 1/2 folded in for the
            # Newton rsqrt iteration below
            ex2h = ep_pool.tile([128, w], F32, tag=f"ex2{tg}")
            nc.vector.tensor_scalar(
                out=ex2h[:], in0=s2h[:], scalar1=0.5 / D, scalar2=0.5 * EPS,
                op0=ALU.mult, op1=ALU.add,
            )
            nmusqh = ep_pool.tile([128, w], F32, tag=f"nmusq{tg}")
            nc.vector.scalar_tensor_tensor(
                out=nmusqh[:], in0=mu[:], scalar=-0.5, in1=mu[:],
                op0=ALU.mult, op1=ALU.mult,
            )
            hvar = ep_pool.tile([128, w], F32, tag=f"var{tg}")
            nc.vector.tensor_tensor(out=hvar[:], in0=nmusqh[:], in1=ex2h[:], op=ALU.add)
            ep_state[tg] = (quad, mu, hvar)

        # rstd = 1/sqrt(2*hvar) entirely on DVE: Quake-style bit seed (magic
        # shifted by 2^22 because we seed from var/2 bits) + two Newton steps
        # y <- y*(1.5 - hvar*y^2); rel err ~2e-5.  No ScalarE involvement, so
        # no cross-engine wait can ever land in front of later squares.
        RSQRT_MAGIC = 0x5F3759DF - 0x00400000
        I32 = mybir.dt.int32

        def ep_rsqrt(tg):
            quad, mu, hvar = ep_state[tg]
            w = hvar.shape[1]
            t1 = ep_pool.tile([128, w], I32, tag=f"rsq_t1{tg}")
            nc.vector.tensor_scalar(
                out=t1[:], in0=hvar[:].bitcast(I32), scalar1=1, scalar2=None,
                op0=ALU.arith_shift_right,
            )
            y = ep_pool.tile([128, w], F32, tag=f"rsq_y{tg}")
            nc.vector.tensor_scalar(
                out=y[:].bitcast(I32), in0=t1[:], scalar1=-1,
                scalar2=RSQRT_MAGIC, op0=ALU.mult, op1=ALU.add,
            )
            for it in range(2):
                y2 = ep_pool.tile([128, w], F32, tag=f"rsq_y2{tg}")
                nc.vector.tensor_tensor(out=y2[:], in0=y[:], in1=y[:], op=ALU.mult)
                nc.vector.tensor_tensor(out=y2[:], in0=y2[:], in1=hvar[:], op=ALU.mult)
                nc.vector.tensor_scalar(
                    out=y2[:], in0=y2[:], scalar1=-1.0, scalar2=1.5,
                    op0=ALU.mult, op1=ALU.add,
                )
                yn = ep_pool.tile([128, w], F32, tag=f"rsq_y{it}{tg}")
                nc.vector.tensor_tensor(out=yn[:], in0=y[:], in1=y2[:], op=ALU.mult)
                y = yn
            ep_state[tg] = (quad, mu, y)

        def ep_dve2(h, a0, a1, tg):
            quad, mu, rstd = ep_state.pop(tg)
            w = a1 - a0
            hs = slice(h * 128 + a0, h * 128 + a1)
            outB = ep_pool.tile([128, w, 2], F32, tag=f"outB{tg}")
            for o, (sw, c) in enumerate(((sw0, c0), (sw1, c1))):
                a = ep_pool.tile([128, w], F32, tag=f"ep_a{tg}")
                # a = p - mu*sW
                nc.vector.scalar_tensor_tensor(
                    out=a[:], in0=mu[:], scalar=-sw, in1=quad[:, :, o],
                    op0=ALU.mult, op1=ALU.add,
                )
                cf = ep_pool.tile([128, w], F32, tag=f"ep_cf{tg}")
                nc.vector.tensor_tensor(
                    out=cf[:], in0=a[:], in1=rstd[:], op=ALU.mult
                )
                # out = (cf + c) * scale   (scale varies along the free axis)
                nc.vector.scalar_tensor_tensor(
                    out=outB[:, :, o], in0=cf[:], scalar=c, in1=sc[:, hs],
                    op0=ALU.add, op1=ALU.mult,
                )
            nc.sync.dma_start(out=out_v[:, hs, :], in_=outB[:])

        def emit_matmuls(j0, xtb):
            """stats matmuls for the quad whose first tile-index is j0."""
            for k in range(QUAD):
                j = j0 + k
                slot = stats_ps[j // 128][:, (j % 128) * 4 : (j % 128) * 4 + 3]
                for c in range(NDCH):
                    nc.tensor.matmul(
                        out=slot,
                        lhsT=xtb[:, k, c, :],
                        rhs=wst[:, c, 0:3],
                        start=(c == 0),
                        stop=(c == NDCH - 1),
                    )

        # Software-pipelined emission: each quad's matmuls are deferred until
        # after the NEXT quad's transposes, so TensorE's in-order queue never
        # sits in a transposes -> (wait for DVE drain) -> matmuls chain; the
        # drain of quad q overlaps the transposes of quad q+1.
        pending = None  # (j0, xtb) of the quad whose matmuls are not yet emitted
        for ci in range(NCHUNKS):
            xf = xf_pool.tile([128, CHUNK, D], BF16)
            if ci == 0:
                qsplit = (2, 4, 4, 6)
            elif ci <= 2 or ci == NCHUNKS - 1:
                # finer DMAs while the HBM pair-contention phase is slow
                # (ramp) and for the last chunk (tail): compute on each
                # quarter starts as soon as that quarter lands
                qsplit = (4, 4, 4, 4)
            else:
                qsplit = (8, 8)
            lo = 0
            for qs in qsplit:
                nc.gpsimd.dma_start(
                    out=xf[:, lo : lo + qs, :],
                    in_=x_v[:, ci * CHUNK + lo : ci * CHUNK + lo + qs, :],
                )
                lo += qs

            for tp in range(CHUNK // QUAD):
                # four token-tiles share one 2-bank bf16 PSUM tile; DVE drains
                # them with a single [128, 2048] 2x-mode tensor_copy
                xtp = ps_pool.tile([128, QUAD, NDCH, 128], BF16)
                xtb = xtb_pool.tile([128, QUAD, NDCH, 128], BF16)
                for k in range(QUAD):
                    t = tp * QUAD + k
                    j = ci * CHUNK + t
                    xt = xf[:, t, :]
                    for c in range(NDCH):
                        nc.tensor.transpose(
                            out=xtp[:, k, c, :],
                            in_=xt[:, c * 128 : (c + 1) * 128],
                            identity=ident[:],
                        )
                    # sum(x^2) needs only xf: emit alongside the transposes.
                    # Chunk 0 gives ScalarE the EARLY tiles so its stream
                    # starts with the first sub-DMA (~4us ramp saving); the
                    # last two chunks shift s2 toward ScalarE since DVE owns
                    # the epilogue-1 tail while ScalarE finishes ~20us early.
                    if ci == 0:
                        on_scalar = t < 10
                    elif ci == NCHUNKS - 1:
                        # DVE owns the epilogue-1 tail; ScalarE (which ends
                        # ~13us earlier) takes every last-chunk square
                        on_scalar = True
                    elif ci == NCHUNKS - 2:
                        on_scalar = t >= 3
                    else:
                        on_scalar = t >= DVE_S2_TILES
                    if on_scalar:
                        nc.scalar.activation(
                            out=act_junk[:], in_=xt, func=AF.Square,
                            accum_out=s2B_act[j // 128][:, j % 128 : j % 128 + 1],
                        )
                    else:
                        nc.vector.scalar_tensor_tensor(
                            out=dve_junk[:], in0=xt,
                            scalar=1.0, in1=xt, op0=ALU.mult, op1=ALU.mult,
                            accum_out=s2B_dve[j // 128][:, j % 128 : j % 128 + 1],
                        )
                nc.vector.tensor_copy(xtb[:], xtp[:])
                if pending is not None:
                    emit_matmuls(*pending)
                pending = (ci * CHUNK + tp * QUAD, xtb)
            # Epilogue 0 (chunks 0-7) is emitted after chunk 12, well into the
            # post-contention phase where DVE runs ahead of ScalarE and all of
            # its inputs are 4+ chunks stale — the var-dependent Sqrt in
            # ScalarE's in-order queue then waits ~1us instead of ~20us (which
            # at chunk 8, during the HBM pair-contention phase, only got
            # absorbed on lucky runs).  Epilogue 1 runs entirely at the end;
            # any earlier placement puts its Sqrt in front of the last chunks'
            # squares with fresh inputs and delays the tail (measured +19us).
            if ci == 12:
                ep_dve1(0, 0, 128, "A")
                ep_rsqrt("A")
                ep_dve2(0, 0, 128, "A")
            elif ci == 14:
                # epilogue 1a (cols 0-95 <- chunks 8-13, matmuls flushed at
                # chunk 14's first quad): all-DVE, so even scheduler hoisting
                # can only cost a small same-engine bubble, not a stall in
                # front of ScalarE's remaining squares
                ep_dve1(1, 0, 96, "B")
                ep_rsqrt("B")
                ep_dve2(1, 0, 96, "B")
            elif ci == NCHUNKS - 1:
                emit_matmuls(*pending)
                pending = None
                # short 32-column tail (chunks 14-15)
                ep_dve1(1, 96, 128, "C")
                ep_rsqrt("C")
                ep_dve2(1, 96, 128, "C")

    _split_multi_waits(nc)
    return nc


def _prepare(x, ln_gamma, ln_beta, W, b, scalers):
    x = np.asarray(x, dtype=np.float32)
    ln_gamma = np.asarray(ln_gamma, dtype=np.float32)
    ln_beta = np.asarray(ln_beta, dtype=np.float32)
    W = np.asarray(W, dtype=np.float32)
    b = np.asarray(b, dtype=np.float32)
    scalers = np.asarray(scalers, dtype=np.float32)

    wg = W * ln_gamma[None, :]                      # [2, 512]
    sw = wg.sum(axis=1)                             # [2]
    c = W @ ln_beta + b                             # [2]
    # wst[k, c, :] = (Wg0[c*128+k], Wg1[c*128+k], 1, 0)
    wst = np.zeros((128, NDCH, 4), dtype=np.float32)
    wst[:, :, 0] = wg[0].reshape(NDCH, 128).T
    wst[:, :, 1] = wg[1].reshape(NDCH, 128).T
    wst[:, :, 2] = 1.0
    wst = np.ascontiguousarray(wst.astype(ml_dtypes.bfloat16))
    ident = np.ascontiguousarray(np.eye(128, dtype=np.float32).astype(ml_dtypes.bfloat16))
    # token t = p*NTILES + j ; subcarrier s = t % 64 = j % 64 (NTILES % 64 == 0)
    scale = np.abs(scalers) + 0.1                   # [64]
    sc_row = scale[(np.arange(NTILES) % S)].astype(np.float32)
    sc_rep = np.ascontiguousarray(
        np.broadcast_to(sc_row[None, :], (128, NTILES))
    )

    nc = _build(float(sw[0]), float(sw[1]), float(c[0]), float(c[1]))

    x_flat = np.ascontiguousarray(x.reshape(TOKENS, D))
    in_maps = [
        {
            "x": x_flat[i * TOK_PER_CORE : (i + 1) * TOK_PER_CORE],
            "ident": ident,
            "wst": wst,
            "sc": sc_rep,
        }
        for i in range(N_CORES)
    ]
    return nc, in_maps


def kernel(x, ln_gamma, ln_beta, W, b, scalers):
    nc, in_maps = _prepare(x, ln_gamma, ln_beta, W, b, scalers)
    res = run_bass_kernel_spmd(nc, in_maps, core_ids=list(range(N_CORES)))
    out = np.concatenate([res.results[i]["out"] for i in range(N_CORES)], axis=0)
    out = np.ascontiguousarray(out.astype(np.float32))
    return out.view(np.complex64).reshape(B, A_BS, A_UE, S)
